# revision 19
# baseline (speedup 1.0000x reference)
"""Trainium2 Bass kernel for y = x @ W^T + b  (B=4096, IN=OUT=2048, fp32).

Sharding: 4-way split on batch x 2-way split on out_features across the 8
NeuronCores.  Each core computes a [1024, 1024] block of the output from
x^T shard [2048, 1024] and W^T shard [2048, 1024] (both pre-transposed and
cast to fp16 on the host: contraction dim on SBUF partitions, contiguous
DMAs, half the HBM traffic of fp32; the PE runs fp16 at the same
1 cycle/row as fp32r, so the matmul stream is unchanged at ~54.6 us).

Schedule (per core), designed so the PE never stalls:
 - ~24 tiny warm-up matmuls on a zeroed scratch tile right after the
   preamble barrier: keeps the PE busy through the HAM activity window
   during the DMA head so the real matmul stream runs at 2.4 GHz (warm)
   from the start instead of paying ~4.5 us of 1.2 GHz cold tax.
 - w k-tiles stream on the SP HWDGE ring (nc.sync), x k-tiles on the ACT
   ring (nc.scalar): two FIFOs, per-transfer overheads overlap, input
   stream stays ahead of the PE.  k0 is split into small first pieces so
   the first real matmul issues ~1 us earlier.
 - Phase A (m-tiles 0-3, all 8 PSUM banks): k-outer loop matching the
   DMA arrival order.
 - Phases B (m 4-6, banks 0-5) and C (m 7, banks 6-7): group-sequential
   (each group runs its 16 k's back to back).  Inputs are all resident
   by then, bank-release waits are met long in advance, and group
   completions stagger 3.4 us apart so the serial DVE bias-add drain
   (0.68 us each) never backs up; the kernel tail is one drain + store.
 - Each [128, 512] half-row stores as soon as its own DVE add is done;
   n0 halves ride the SP ring (idle after the w stream), n1 halves the
   ACT ring; the very last half-row splits across both rings.

Constraint driving the sync passes below: a Matmult on TRN2 supports
only ONE sync-wait; Tile can emit more, so extra waits are legalized
into EventSemaphore prefixes on the issuing engine.
"""

import os

import numpy as np

P = 128
B, IN, OUT = 4096, 2048, 2048
MB_SPLIT, NB_SPLIT = 4, 2  # batch-split x out-split = 8 cores
BM = B // MB_SPLIT  # 1024 batch rows per core
NO = OUT // NB_SPLIT  # 1024 out cols per core
KT = IN // P  # 16 k-tiles
MT = BM // P  # 8 m-tiles
NFREE = 512  # PSUM bank free dim (fp32)
NT = NO // NFREE  # 2 n-tiles
N_CORES = 8
X0A = 256  # first x piece: m-tiles 0-1 of k0

N_WARMUP = int(os.environ.get("BASS_N_WARMUP", "14"))
WARM_N = int(os.environ.get("BASS_WARM_N", "256"))
MM_DT = os.environ.get("BASS_MM_DT", "float16")

_CACHE = {}


def _np_in_dtype(mm_dt_name: str):
    if mm_dt_name == "float16":
        return np.float16
    if mm_dt_name == "bfloat16":
        import ml_dtypes

        return ml_dtypes.bfloat16
    return np.float32


def _build(mm_dt_name: str):
    import concourse.bass as bass
    import concourse.mybir as mybir
    import concourse.tile as tile

    mmdt = getattr(mybir.dt, mm_dt_name)
    f32 = mybir.dt.float32

    nc = bass.Bass("TRN2", target_bir_lowering=False, debug=False,
                   num_devices=N_CORES)
    xt = nc.dram_tensor("xt", [IN, BM], mmdt, kind="ExternalInput")
    wt = nc.dram_tensor("wt", [IN, NO], mmdt, kind="ExternalInput")
    bi = nc.dram_tensor("bi", [NO], f32, kind="ExternalInput")
    y = nc.dram_tensor("y", [BM, NO], f32, kind="ExternalOutput")

    xt_r = xt.ap().rearrange("(k p) m -> p k m", p=P)  # [128, 16, 1024]
    wt_r = wt.ap().rearrange("(k p) n -> p k n", p=P)
    y_ap = y.ap()

    with tile.TileContext(nc) as tc:
        with (
            tc.tile_pool(name="xp", bufs=1) as xp,
            tc.tile_pool(name="wp", bufs=1) as wp,
            tc.tile_pool(name="bp", bufs=1) as bp,
            tc.tile_pool(name="op", bufs=1) as op,
            tc.tile_pool(name="sc", bufs=1) as sc,
            tc.tile_pool(name="ps", bufs=1, space="PSUM") as ps,
        ):
            # --- PE warm-up: matmuls on a raw (non-pool) scratch SBUF
            # tensor, contents irrelevant and results discarded.  Raw so
            # there is no memset/write dependency: the PE starts the
            # moment its preamble ends, keeping it busy through the HAM
            # activity window while the first input tiles stream in. ---
            scratch = nc.alloc_sbuf_tensor("warm_scratch", [P, WARM_N],
                                           mmdt)
            warm_ps = ps.tile([P, NFREE], f32, tag="ps0", name="warm_ps")
            for i in range(N_WARMUP):
                nc.tensor.matmul(
                    warm_ps[:, :WARM_N], lhsT=scratch.ap()[:, :P],
                    rhs=scratch.ap()[:], start=True, stop=True,
                    skip_group_check=True)

            # --- input DMA emission: w k-tiles on the SP ring, x
            # k-tiles on the ACT ring.  x is split per phase (xa: m 0-3
            # for phase A, xb: m 4-7 streamed after all xa) and w0 per
            # n-half, so the first matmul's dependencies are two 128KB
            # tiles and phase A never waits on phase-B/C bytes. ---
            bias_sb = bp.tile([P, NO], f32, tag="bias")
            wk = [None] * KT
            xk = [None] * KT
            HALF = BM // 2
            w0 = [None, None]
            for n in range(NT):
                t = wp.tile([P, NFREE], mmdt, tag=f"w0_{n}", name=f"w0_{n}")
                nc.sync.dma_start(t[:], wt_r[:, 0, n * NFREE:(n + 1) * NFREE])
                w0[n] = t
            xa0 = xp.tile([P, HALF], mmdt, tag="xa0", name="xa0")
            nc.scalar.dma_start(xa0[:], xt_r[:, 0, :HALF])
            for k in range(1, KT):
                t = wp.tile([P, NO], mmdt, tag=f"wk{k}", name=f"wk{k}")
                nc.sync.dma_start(t[:], wt_r[:, k, :])
                wk[k] = t
                t = xp.tile([P, BM], mmdt, tag=f"xk{k}", name=f"xk{k}")
                nc.scalar.dma_start(t[:], xt_r[:, k, :])
                xk[k] = t
                if k == 13:
                    nc.sync.dma_start(
                        bias_sb[:], bi.ap()[None, :].to_broadcast((P, NO)))
            xb0 = xp.tile([P, HALF], mmdt, tag="xb0", name="xb0")
            nc.scalar.dma_start(xb0[:], xt_r[:, 0, HALF:])

            def get_x(k, mt):  # lhsT slice for absolute m-tile mt
                if k == 0:
                    if mt < 4:
                        return xa0[:, mt * P:(mt + 1) * P]
                    return xb0[:, (mt - 4) * P:(mt - 3) * P]
                return xk[k][:, mt * P:(mt + 1) * P]

            def get_w(k, n):
                if k == 0:
                    return w0[n][:]
                return wk[k][:, n * NFREE:(n + 1) * NFREE]

            psum = {}

            def mm(k, mt, n):
                nc.tensor.matmul(
                    psum[(mt, n)][:], lhsT=get_x(k, mt), rhs=get_w(k, n),
                    start=(k == 0), stop=(k == KT - 1),
                    skip_group_check=(k == 0))

            def drain(mt, n, split_store=False):
                ot = op.tile([P, NFREE], f32, tag=f"out{mt}_{n}",
                             name=f"out_{mt}_{n}")
                nc.vector.tensor_add(
                    ot[:], psum[(mt, n)][:],
                    bias_sb[:, n * NFREE:(n + 1) * NFREE])
                row0 = mt * P
                col0 = n * NFREE
                if split_store:
                    h = NFREE // 2
                    nc.sync.dma_start(
                        y_ap[row0:row0 + P, col0:col0 + h], ot[:, :h])
                    nc.scalar.dma_start(
                        y_ap[row0:row0 + P, col0 + h:col0 + NFREE], ot[:, h:])
                else:
                    eng = nc.sync if n == 0 else nc.scalar
                    eng.dma_start(
                        y_ap[row0:row0 + P, col0:col0 + NFREE], ot[:])

            # --- Phase A: m 0-3, k-outer, all 8 banks ---
            groups_a = [(m, n) for m in range(4) for n in range(NT)]
            for gi, (m, n) in enumerate(groups_a):
                psum[(m, n)] = ps.tile([P, NFREE], f32, tag=f"ps{gi}",
                                       name=f"psum_a_{gi}")
            # k0 n-major: the four n0 matmuls only need w0_0, giving
            # w0_1 an extra ~1.7us to arrive
            for m in range(4):
                mm(0, m, 0)
            for m in range(4):
                mm(0, m, 1)
            for k in range(1, KT):
                for m, n in groups_a:
                    mm(k, m, n)
            for m, n in groups_a:
                drain(m, n)

            # --- Phases B (m 4-6, banks 0-5) and C (m 7, banks 6-7):
            # group-sequential ---
            groups_bc = [(m, n) for m in range(4, MT) for n in range(NT)]
            for gi, (m, n) in enumerate(groups_bc):
                psum[(m, n)] = ps.tile([P, NFREE], f32, tag=f"ps{gi}",
                                       name=f"psum_bc_{gi}")
            for m, n in groups_bc:
                for k in range(KT):
                    mm(k, m, n)
                drain(m, n, split_store=(m == MT - 1 and n == NT - 1))

    _strip_redundant_pe_waits(nc)
    _legalize_multi_waits(nc)
    _check_matmul_waits(nc)
    return nc


def _legalize_multi_waits(nc):
    """Split multi-wait instructions into single-wait EventSemaphore
    prefixes on the same engine.

    This walrus pipeline (bass pass list, no lower_sync) supports exactly
    ONE sync wait per instruction.  A chain of EventSemaphore waits on the
    issuing engine followed by the instruction with the final wait is
    semantically identical: the engine's sequencer blocks on each in
    order.
    """
    import copy

    import concourse.mybir as mybir

    m = nc.m
    new_module = copy.replace(m, functions=[])
    counter = [0]
    for function in m.functions:
        new_function = copy.replace(function, blocks=[])
        new_function.set_allocations_from_list(function.allocations)
        for block in function.blocks:
            new_insts = []
            for inst in block.instructions:
                s = inst.sync_info
                if s and s.on_wait and len(s.on_wait) > 1:
                    for w in s.on_wait[:-1]:
                        counter[0] += 1
                        ev = mybir.InstEventSemaphore(
                            name=f"legalize_wait_{counter[0]}",
                            ins=[], outs=[],
                            sync_info=mybir.SyncInfo(on_wait=[w],
                                                     on_update=[]),
                            engine=inst.engine,
                        )
                        new_insts.append(ev)
                    inst.sync_info = mybir.SyncInfo(
                        on_wait=[s.on_wait[-1]], on_update=s.on_update)
                new_insts.append(inst)
            new_function.blocks.append(
                copy.replace(block, instructions=new_insts))
        new_module.functions.append(new_function)
    nc.m = new_module


def _strip_redundant_pe_waits(nc):
    """Drop PE self-waits on matmuls that also wait on the DVE release.

    TRN2 matmuls support one sync wait.  Tile's wait emission is not
    transitively minimal: a PSUM-bank reuse emits both the bank's last PE
    writer (self-engine, redundant: the DVE add that releases the bank
    already waits on that writer) and the DVE release.  Keeping the DVE
    wait preserves the hazard ordering.
    """
    import concourse.mybir as mybir

    for bb in nc.m.functions[0].blocks:
        for inst in bb.instructions:
            if type(inst).__name__ != "InstMatmult":
                continue
            s = inst.sync_info
            if not (s and s.on_wait and len(s.on_wait) > 1):
                continue
            keep = [w for w in s.on_wait if not w.ant_name.startswith("PE")]
            dve = [w for w in keep if w.ant_name.startswith("DVE")]
            if len(keep) == len(s.on_wait) - 1 and dve:
                inst.sync_info = mybir.SyncInfo(on_wait=keep,
                                                on_update=s.on_update)


def _check_matmul_waits(nc):
    """TRN2 compute instructions (Matmult, TensorTensor, ...) support one
    sync wait; walrus codegen hard-fails on more."""
    limited = {"InstMatmult", "InstTensorTensor", "InstTensorScalarPtr",
               "InstActivation", "InstTensorCopy", "InstCopy"}
    bad = []
    for bb in nc.m.functions[0].blocks:
        for inst in bb.instructions:
            if type(inst).__name__ in limited:
                s = inst.sync_info
                nw = len(s.on_wait) if s and s.on_wait else 0
                if nw > 1:
                    bad.append((inst.name, type(inst).__name__,
                                [(w.ant_name, w.wait_value)
                                 for w in s.on_wait]))
    if bad:
        raise RuntimeError(f"{len(bad)} insts with >1 wait: {bad[:8]}")


def make_in_maps(x, weights, bias, mm_dt_name=None):
    """Host-side shard + transpose + cast for the 8 cores."""
    mm_dt_name = mm_dt_name or MM_DT
    in_dt = _np_in_dtype(mm_dt_name)
    xT = np.ascontiguousarray(x.T.astype(in_dt))  # [IN, B]
    wT = np.ascontiguousarray(weights.T.astype(in_dt))  # [IN, OUT]
    bias = np.asarray(bias, dtype=np.float32)

    in_maps = []
    for c in range(N_CORES):
        mb, nb = divmod(c, NB_SPLIT)
        in_maps.append({
            "xt": np.ascontiguousarray(xT[:, mb * BM:(mb + 1) * BM]),
            "wt": np.ascontiguousarray(wT[:, nb * NO:(nb + 1) * NO]),
            "bi": np.ascontiguousarray(bias[nb * NO:(nb + 1) * NO]),
        })
    return in_maps


def kernel(x, weights, bias):
    from concourse.bass_utils import run_bass_kernel_spmd

    x = np.asarray(x, dtype=np.float32)
    weights = np.asarray(weights, dtype=np.float32)
    bias = np.asarray(bias, dtype=np.float32)

    if MM_DT not in _CACHE:
        _CACHE[MM_DT] = _build(MM_DT)
    nc = _CACHE[MM_DT]

    in_maps = make_in_maps(x, weights, bias, MM_DT)
    res = run_bass_kernel_spmd(nc, in_maps, core_ids=list(range(N_CORES)))

    out = np.empty((B, OUT), dtype=np.float32)
    for c in range(N_CORES):
        mb, nb = divmod(c, NB_SPLIT)
        out[mb * BM:(mb + 1) * BM, nb * NO:(nb + 1) * NO] = res.results[c]["y"]
    return out


# revision 20
# speedup vs baseline: 1.0331x; 1.0331x over previous
"""Trainium2 Bass kernel for y = x @ W^T + b  (B=4096, IN=OUT=2048, fp32).

Sharding: 4-way split on batch x 2-way split on out_features across the 8
NeuronCores.  Each core computes a [1024, 1024] block of the output from
x^T shard [2048, 1024] and W^T shard [2048, 1024] (both pre-transposed and
cast to fp16 on the host: contraction dim on SBUF partitions, contiguous
DMAs, half the HBM traffic of fp32; the PE runs fp16 at the same
1 cycle/row as fp32r, so the matmul stream is unchanged at ~54.6 us).

Schedule (per core), designed so the PE never stalls:
 - ~24 tiny warm-up matmuls on a zeroed scratch tile right after the
   preamble barrier: keeps the PE busy through the HAM activity window
   during the DMA head so the real matmul stream runs at 2.4 GHz (warm)
   from the start instead of paying ~4.5 us of 1.2 GHz cold tax.
 - w k-tiles stream on the SP HWDGE ring (nc.sync), x k-tiles on the ACT
   ring (nc.scalar): two FIFOs, per-transfer overheads overlap, input
   stream stays ahead of the PE.  k0 is split into small first pieces so
   the first real matmul issues ~1 us earlier.
 - Phase A (m-tiles 0-3, all 8 PSUM banks): k-outer loop matching the
   DMA arrival order.
 - Phases B (m 4-6, banks 0-5) and C (m 7, banks 6-7): group-sequential
   (each group runs its 16 k's back to back).  Inputs are all resident
   by then, bank-release waits are met long in advance, and group
   completions stagger 3.4 us apart so the serial DVE bias-add drain
   (0.68 us each) never backs up; the kernel tail is one drain + store.
 - Each [128, 512] half-row stores as soon as its own DVE add is done;
   n0 halves ride the SP ring (idle after the w stream), n1 halves the
   ACT ring; the very last half-row splits across both rings.

Constraint driving the sync passes below: a Matmult on TRN2 supports
only ONE sync-wait; Tile can emit more, so extra waits are legalized
into EventSemaphore prefixes on the issuing engine.
"""

import os

import numpy as np

P = 128
B, IN, OUT = 4096, 2048, 2048
MB_SPLIT, NB_SPLIT = 4, 2  # batch-split x out-split = 8 cores
BM = B // MB_SPLIT  # 1024 batch rows per core
NO = OUT // NB_SPLIT  # 1024 out cols per core
KT = IN // P  # 16 k-tiles
MT = BM // P  # 8 m-tiles
NFREE = 512  # PSUM bank free dim (fp32)
NT = NO // NFREE  # 2 n-tiles
N_CORES = 8
X0A = 256  # first x piece: m-tiles 0-1 of k0

N_WARMUP = int(os.environ.get("BASS_N_WARMUP", "18"))
WARM_N = int(os.environ.get("BASS_WARM_N", "256"))
MM_DT = os.environ.get("BASS_MM_DT", "float16")

_CACHE = {}


def _np_in_dtype(mm_dt_name: str):
    if mm_dt_name == "float16":
        return np.float16
    if mm_dt_name == "bfloat16":
        import ml_dtypes

        return ml_dtypes.bfloat16
    return np.float32


def _build(mm_dt_name: str):
    import concourse.bass as bass
    import concourse.mybir as mybir
    import concourse.tile as tile

    mmdt = getattr(mybir.dt, mm_dt_name)
    f32 = mybir.dt.float32

    nc = bass.Bass("TRN2", target_bir_lowering=False, debug=False,
                   num_devices=N_CORES)
    xt = nc.dram_tensor("xt", [IN, BM], mmdt, kind="ExternalInput")
    wt = nc.dram_tensor("wt", [IN, NO], mmdt, kind="ExternalInput")
    bi = nc.dram_tensor("bi", [NO], f32, kind="ExternalInput")
    y = nc.dram_tensor("y", [BM, NO], f32, kind="ExternalOutput")

    xt_r = xt.ap().rearrange("(k p) m -> p k m", p=P)  # [128, 16, 1024]
    wt_r = wt.ap().rearrange("(k p) n -> p k n", p=P)
    y_ap = y.ap()

    with tile.TileContext(nc) as tc:
        with (
            tc.tile_pool(name="xp", bufs=1) as xp,
            tc.tile_pool(name="wp", bufs=1) as wp,
            tc.tile_pool(name="bp", bufs=1) as bp,
            tc.tile_pool(name="op", bufs=1) as op,
            tc.tile_pool(name="sc", bufs=1) as sc,
            tc.tile_pool(name="ps", bufs=1, space="PSUM") as ps,
        ):
            # --- PE warm-up: matmuls on a raw (non-pool) scratch SBUF
            # tensor, contents irrelevant and results discarded.  Raw so
            # there is no memset/write dependency: the PE starts the
            # moment its preamble ends, keeping it busy through the HAM
            # activity window while the first input tiles stream in. ---
            scratch = nc.alloc_sbuf_tensor("warm_scratch", [P, WARM_N],
                                           mmdt)
            warm_ps = ps.tile([P, NFREE], f32, tag="ps0", name="warm_ps")
            for i in range(N_WARMUP):
                nc.tensor.matmul(
                    warm_ps[:, :WARM_N], lhsT=scratch.ap()[:, :P],
                    rhs=scratch.ap()[:], start=True, stop=True,
                    skip_group_check=True)

            # --- input DMA emission: w k-tiles on the SP ring, x
            # k-tiles on the ACT ring.  x is split per phase (xa: m 0-3
            # for phase A, xb: m 4-7 streamed after all xa) and w0 per
            # n-half, so the first matmul's dependencies are two 128KB
            # tiles and phase A never waits on phase-B/C bytes. ---
            bias_sb = bp.tile([P, NO], f32, tag="bias")
            wk = [None] * KT
            xk = [None] * KT
            HALF = BM // 2
            w0 = [None, None]
            for n in range(NT):
                t = wp.tile([P, NFREE], mmdt, tag=f"w0_{n}", name=f"w0_{n}")
                nc.sync.dma_start(t[:], wt_r[:, 0, n * NFREE:(n + 1) * NFREE])
                w0[n] = t
            xa0 = xp.tile([P, HALF], mmdt, tag="xa0", name="xa0")
            nc.scalar.dma_start(xa0[:], xt_r[:, 0, :HALF])
            for k in range(1, KT):
                t = wp.tile([P, NO], mmdt, tag=f"wk{k}", name=f"wk{k}")
                nc.sync.dma_start(t[:], wt_r[:, k, :])
                wk[k] = t
                t = xp.tile([P, BM], mmdt, tag=f"xk{k}", name=f"xk{k}")
                nc.scalar.dma_start(t[:], xt_r[:, k, :])
                xk[k] = t
                if k == 13:
                    nc.sync.dma_start(
                        bias_sb[:], bi.ap()[None, :].to_broadcast((P, NO)))
            xb0 = xp.tile([P, HALF], mmdt, tag="xb0", name="xb0")
            nc.scalar.dma_start(xb0[:], xt_r[:, 0, HALF:])

            def get_x(k, mt):  # lhsT slice for absolute m-tile mt
                if k == 0:
                    if mt < 4:
                        return xa0[:, mt * P:(mt + 1) * P]
                    return xb0[:, (mt - 4) * P:(mt - 3) * P]
                return xk[k][:, mt * P:(mt + 1) * P]

            def get_w(k, n):
                if k == 0:
                    return w0[n][:]
                return wk[k][:, n * NFREE:(n + 1) * NFREE]

            psum = {}

            def mm(k, mt, n):
                nc.tensor.matmul(
                    psum[(mt, n)][:], lhsT=get_x(k, mt), rhs=get_w(k, n),
                    start=(k == 0), stop=(k == KT - 1),
                    skip_group_check=(k == 0))

            def drain(mt, n, split_store=False):
                ot = op.tile([P, NFREE], f32, tag=f"out{mt}_{n}",
                             name=f"out_{mt}_{n}")
                nc.vector.tensor_add(
                    ot[:], psum[(mt, n)][:],
                    bias_sb[:, n * NFREE:(n + 1) * NFREE])
                row0 = mt * P
                col0 = n * NFREE
                if split_store:
                    h = NFREE // 2
                    nc.sync.dma_start(
                        y_ap[row0:row0 + P, col0:col0 + h], ot[:, :h])
                    nc.scalar.dma_start(
                        y_ap[row0:row0 + P, col0 + h:col0 + NFREE], ot[:, h:])
                else:
                    eng = nc.sync if n == 0 else nc.scalar
                    eng.dma_start(
                        y_ap[row0:row0 + P, col0:col0 + NFREE], ot[:])

            # --- Phase A: m 0-3, k-outer, all 8 banks ---
            groups_a = [(m, n) for m in range(4) for n in range(NT)]
            for gi, (m, n) in enumerate(groups_a):
                psum[(m, n)] = ps.tile([P, NFREE], f32, tag=f"ps{gi}",
                                       name=f"psum_a_{gi}")
            # k0 n-major: the four n0 matmuls only need w0_0, giving
            # w0_1 an extra ~1.7us to arrive
            for m in range(4):
                mm(0, m, 0)
            for m in range(4):
                mm(0, m, 1)
            for k in range(1, KT):
                for m, n in groups_a:
                    mm(k, m, n)
            for m, n in groups_a:
                drain(m, n)

            # --- Phases B (m 4-6, banks 0-5) and C (m 7, banks 6-7):
            # group-sequential ---
            groups_bc = [(m, n) for m in range(4, MT) for n in range(NT)]
            for gi, (m, n) in enumerate(groups_bc):
                psum[(m, n)] = ps.tile([P, NFREE], f32, tag=f"ps{gi}",
                                       name=f"psum_bc_{gi}")
            for m, n in groups_bc:
                for k in range(KT):
                    mm(k, m, n)
                drain(m, n, split_store=(m == MT - 1 and n == NT - 1))

    _strip_redundant_pe_waits(nc)
    _legalize_multi_waits(nc)
    _check_matmul_waits(nc)
    return nc


def _legalize_multi_waits(nc):
    """Split multi-wait instructions into single-wait EventSemaphore
    prefixes on the same engine.

    This walrus pipeline (bass pass list, no lower_sync) supports exactly
    ONE sync wait per instruction.  A chain of EventSemaphore waits on the
    issuing engine followed by the instruction with the final wait is
    semantically identical: the engine's sequencer blocks on each in
    order.
    """
    import copy

    import concourse.mybir as mybir

    m = nc.m
    new_module = copy.replace(m, functions=[])
    counter = [0]
    for function in m.functions:
        new_function = copy.replace(function, blocks=[])
        new_function.set_allocations_from_list(function.allocations)
        for block in function.blocks:
            new_insts = []
            for inst in block.instructions:
                s = inst.sync_info
                if s and s.on_wait and len(s.on_wait) > 1:
                    for w in s.on_wait[:-1]:
                        counter[0] += 1
                        ev = mybir.InstEventSemaphore(
                            name=f"legalize_wait_{counter[0]}",
                            ins=[], outs=[],
                            sync_info=mybir.SyncInfo(on_wait=[w],
                                                     on_update=[]),
                            engine=inst.engine,
                        )
                        new_insts.append(ev)
                    inst.sync_info = mybir.SyncInfo(
                        on_wait=[s.on_wait[-1]], on_update=s.on_update)
                new_insts.append(inst)
            new_function.blocks.append(
                copy.replace(block, instructions=new_insts))
        new_module.functions.append(new_function)
    nc.m = new_module


def _strip_redundant_pe_waits(nc):
    """Drop PE self-waits on matmuls that also wait on the DVE release.

    TRN2 matmuls support one sync wait.  Tile's wait emission is not
    transitively minimal: a PSUM-bank reuse emits both the bank's last PE
    writer (self-engine, redundant: the DVE add that releases the bank
    already waits on that writer) and the DVE release.  Keeping the DVE
    wait preserves the hazard ordering.
    """
    import concourse.mybir as mybir

    for bb in nc.m.functions[0].blocks:
        for inst in bb.instructions:
            if type(inst).__name__ != "InstMatmult":
                continue
            s = inst.sync_info
            if not (s and s.on_wait and len(s.on_wait) > 1):
                continue
            keep = [w for w in s.on_wait if not w.ant_name.startswith("PE")]
            dve = [w for w in keep if w.ant_name.startswith("DVE")]
            if len(keep) == len(s.on_wait) - 1 and dve:
                inst.sync_info = mybir.SyncInfo(on_wait=keep,
                                                on_update=s.on_update)


def _check_matmul_waits(nc):
    """TRN2 compute instructions (Matmult, TensorTensor, ...) support one
    sync wait; walrus codegen hard-fails on more."""
    limited = {"InstMatmult", "InstTensorTensor", "InstTensorScalarPtr",
               "InstActivation", "InstTensorCopy", "InstCopy"}
    bad = []
    for bb in nc.m.functions[0].blocks:
        for inst in bb.instructions:
            if type(inst).__name__ in limited:
                s = inst.sync_info
                nw = len(s.on_wait) if s and s.on_wait else 0
                if nw > 1:
                    bad.append((inst.name, type(inst).__name__,
                                [(w.ant_name, w.wait_value)
                                 for w in s.on_wait]))
    if bad:
        raise RuntimeError(f"{len(bad)} insts with >1 wait: {bad[:8]}")


def make_in_maps(x, weights, bias, mm_dt_name=None):
    """Host-side shard + transpose + cast for the 8 cores."""
    mm_dt_name = mm_dt_name or MM_DT
    in_dt = _np_in_dtype(mm_dt_name)
    xT = np.ascontiguousarray(x.T.astype(in_dt))  # [IN, B]
    wT = np.ascontiguousarray(weights.T.astype(in_dt))  # [IN, OUT]
    bias = np.asarray(bias, dtype=np.float32)

    in_maps = []
    for c in range(N_CORES):
        mb, nb = divmod(c, NB_SPLIT)
        in_maps.append({
            "xt": np.ascontiguousarray(xT[:, mb * BM:(mb + 1) * BM]),
            "wt": np.ascontiguousarray(wT[:, nb * NO:(nb + 1) * NO]),
            "bi": np.ascontiguousarray(bias[nb * NO:(nb + 1) * NO]),
        })
    return in_maps


def kernel(x, weights, bias):
    from concourse.bass_utils import run_bass_kernel_spmd

    x = np.asarray(x, dtype=np.float32)
    weights = np.asarray(weights, dtype=np.float32)
    bias = np.asarray(bias, dtype=np.float32)

    if MM_DT not in _CACHE:
        _CACHE[MM_DT] = _build(MM_DT)
    nc = _CACHE[MM_DT]

    in_maps = make_in_maps(x, weights, bias, MM_DT)
    res = run_bass_kernel_spmd(nc, in_maps, core_ids=list(range(N_CORES)))

    out = np.empty((B, OUT), dtype=np.float32)
    for c in range(N_CORES):
        mb, nb = divmod(c, NB_SPLIT)
        out[mb * BM:(mb + 1) * BM, nb * NO:(nb + 1) * NO] = res.results[c]["y"]
    return out
